# revision 20
# baseline (speedup 1.0000x reference)
"""NeuralGCDE Trainium2 kernel (bf16 rewrite).

Strategy: data-parallel over batch B=32 across 8 NeuronCores (B_loc=4 per
core, graph supports/weights replicated, zero inter-core communication).
Per core, the RK4 time scan (12 steps x 4 stages) runs fully on-device.

Layouts (per core, tokens tok = b*256+n, 1024 tokens, 2 chunks of 512):
  - state hz [128, 1024] bf16: partition p = 64*chunk + feature,
    cols 0:512 = h (token within chunk), cols 512:1024 = z.
  - XG [128 (k*64+i), 1024 tok]: graph-conv input (k=0: x, k=1: A@x)
  - xt[mi] [128 (node m within half mi), 4b*64+i]: token-major x for the
    support matmul, produced DIRECTLY from z by 8 small matmuls
    (stationary = z-slices, moving = Wg_in) -- no PE transposes.
  - adaptive per-node weights factorized through the embedding (EMB=10):
    U_c = wp_c.T @ XG; V_c = U_c .* Eg-mask (vector/gpsimd); the
    d-reduction and output projection fused into one accumulating
    matmul chain over WGOUTD; node bias via (b_pool@Wg_out).T @ EGU.
All compute bf16 (PSUM fp32); RK state kept in bf16 with one-op on-chain
updates (off-chain partial combos overlap the previous RK stage).
"""
import sys
import os
import numpy as np
import ml_dtypes

if "/opt/trn_rl_repo" not in sys.path:
    sys.path.insert(0, "/opt/trn_rl_repo")

BF16NP = ml_dtypes.bfloat16

B, N, T, CIN, HID, EMB, KCH = 32, 256, 13, 2, 64, 10, 2
NCORES = 8
BLOC = B // NCORES          # 4
TOK = BLOC * N              # 1024
NSTEP = T - 1               # 12
NSTAGE = 3 * NSTEP + 1      # 37 distinct spline-derivative tensors

_KERNEL_CACHE = {}
LAST_RES = None  # BassKernelResults of the most recent run (for test.py)


def _dx_stage_index(t, s):
    """Index into the 37-entry dX table for RK stage s of step t."""
    if s < 3:
        return 3 * t + s
    return 3 * (t + 1) if (t + 1) < NSTEP else 3 * NSTEP


def _build(n_steps=NSTEP):
    import concourse.bacc as bacc
    import concourse.tile as tile
    from concourse import mybir
    from contextlib import ExitStack

    F32 = mybir.dt.float32
    F32R = mybir.dt.float32r
    BF16 = mybir.dt.bfloat16
    AF = mybir.ActivationFunctionType
    ALU = mybir.AluOpType

    nc = bacc.Bacc("TRN2", target_bir_lowering=False, debug=False,
                   num_devices=NCORES)

    def din(name, shape, dt=BF16):
        return nc.dram_tensor(name, shape, dt, kind="ExternalInput").ap()

    H0Z0 = din("H0Z0", [128, 1024], mybir.dt.float32r)
    WFIN = din("WFIN", [128, 128])        # block-diag dup of Wf_in
    WFHID = din("WFHID", [128, 128])
    WGINF = din("WGINF", [128, 128])      # block-diag dup of Wg_in
    WFINR = din("WFINR", [128, 128], mybir.dt.float32r)
    WGINFR = din("WGINFR", [128, 128], mybir.dt.float32r)
    WFOUT_A = din("WFOUT_A", [128, 128])  # [Wf_out_perm; 0]
    WFOUT_B = din("WFOUT_B", [128, 128])  # [0; Wf_out_perm]
    WGOUTD = din("WGOUTD", [128, 128])    # [Wg_out_perm; Wg_out_perm]
    BP2 = din("BP2", [10, 128])           # b_pool @ Wg_out_perm
    BFIN2 = din("BFIN2", [128, 1], F32)
    BFHID2 = din("BFHID2", [128, 1], F32)
    BGIN2 = din("BGIN2", [128, 1], F32)
    BFOUT = din("BFOUT", [128, 1], F32)   # i-major permuted
    BGOUT = din("BGOUT", [128, 1], F32)
    IDENT = din("IDENT", [64, 64])
    AT0 = din("AT0", [128, 256])          # A.T rows 0:128
    AT1 = din("AT1", [128, 256])
    WP = din("WP", [128, 640])            # [k*64+i, d*64+o]
    EGU = din("EGU", [10, 1024])          # Eg[n(tok), d]
    EGT = [din(f"EGT{c}", [128, 1024]) for c in range(5)]
    DXB = din("DXB", [NSTAGE, 128, 1024])
    ZOUT = nc.dram_tensor("ZOUT", [NSTEP, 128, 512], mybir.dt.float32r,
                          kind="ExternalOutput").ap()

    with tile.TileContext(nc) as tc, ExitStack() as ctx:
        cp = ctx.enter_context(tc.tile_pool(name="const", bufs=1))
        st = ctx.enter_context(tc.tile_pool(name="state", bufs=2))
        kp = ctx.enter_context(tc.tile_pool(name="kpool", bufs=2))
        wk = ctx.enter_context(tc.tile_pool(name="work", bufs=2))
        vp = ctx.enter_context(tc.tile_pool(name="vpool", bufs=3))
        dxp = ctx.enter_context(tc.tile_pool(name="dxp", bufs=3))
        psA = ctx.enter_context(tc.tile_pool(name="psA", bufs=2, space="PSUM"))
        psXT = ctx.enter_context(tc.tile_pool(name="psXT", bufs=1, space="PSUM"))
        psS = ctx.enter_context(tc.tile_pool(name="psS", bufs=1, space="PSUM"))
        psU = ctx.enter_context(tc.tile_pool(name="psU", bufs=1, space="PSUM"))
        psG = ctx.enter_context(tc.tile_pool(name="psG", bufs=1, space="PSUM"))

        # ---- resident constants ----
        def cload(src, shape, tag, dt=BF16):
            t = cp.tile(shape, dt, tag=tag)
            nc.sync.dma_start(t[:], src)
            return t

        wfin = cload(WFIN, [128, 128], "wfin")
        wfhid = cload(WFHID, [128, 128], "wfhid")
        wginf = cload(WGINF, [128, 128], "wginf")
        wfinr = cload(WFINR, [128, 128], "wfinr", F32R)
        wginfr = cload(WGINFR, [128, 128], "wginfr", F32R)
        wfout_a = cload(WFOUT_A, [128, 128], "wfout_a")
        wfout_b = cload(WFOUT_B, [128, 128], "wfout_b")
        wgoutd = cload(WGOUTD, [128, 128], "wgoutd")
        bp2 = cload(BP2, [10, 128], "bp2")
        bfin2 = cload(BFIN2, [128, 1], "bfin2", F32)
        bfhid2 = cload(BFHID2, [128, 1], "bfhid2", F32)
        bgin2 = cload(BGIN2, [128, 1], "bgin2", F32)
        bfout = cload(BFOUT, [128, 1], "bfout", F32)
        bgout = cload(BGOUT, [128, 1], "bgout", F32)
        ident = cload(IDENT, [64, 64], "ident")
        at0 = cload(AT0, [128, 256], "at0")
        at1 = cload(AT1, [128, 256], "at1")
        wp = cload(WP, [128, 640], "wp")
        egu = cload(EGU, [10, 1024], "egu")
        egt = [cload(EGT[c], [128, 1024], f"egt{c}") for c in range(5)]

        # ---- state ----
        hz = st.tile([128, 1024], F32R, tag="hz")
        nc.sync.dma_start(hz[:], H0Z0)

        def vf(hzin, sidx, k_tag, head_f, head_g):
            """One vector-field eval -> khz [128,1024] bf16 (kh|kz)."""
            dxf = dxp.tile([128, 1024], BF16, tag="dxf")
            nc.sync.dma_start(dxf[:], DXB[sidx])

            # ---- PE head: pg1, pf1 ----
            pg1 = psA.tile([128, 512], F32, tag="m")
            nc.tensor.matmul(pg1[:], head_g[:], hzin[:, 512:1024],
                             start=True, stop=True)
            pf1 = psA.tile([128, 512], F32, tag="m")
            nc.tensor.matmul(pf1[:], head_f[:], hzin[:, 0:512],
                             start=True, stop=True)

            # XG rows 0:64 = x = relu(pg1 + bg_in)  (scalar + vector halves)
            XG = wk.tile([128, 1024], BF16, tag="XG")
            nc.scalar.activation(XG[0:64, 0:512], pg1[0:64, :], AF.Relu,
                                 bias=bgin2[0:64], scale=1.0)
            nc.vector.tensor_scalar(XG[0:64, 512:1024], pg1[64:128, :],
                                    bgin2[64:128], 0.0, ALU.add, ALU.max)

            # xt via PE transposes of x: xt cols mi*256 + b*64 + i
            pxt = psXT.tile([128, 512], BF16, tag="pxt")
            for mi in range(2):
                for b in range(BLOC):
                    nc.tensor.transpose(
                        pxt[:, mi * 256 + b * 64: mi * 256 + (b + 1) * 64],
                        XG[0:64, b * 256 + mi * 128: b * 256 + (mi + 1) * 128],
                        ident[:],
                    )
            xt = wk.tile([128, 512], BF16, tag="xt")
            nc.vector.tensor_copy(xt[:, 0:256], pxt[:, 0:256])
            nc.scalar.copy(xt[:, 256:512], pxt[:, 256:512])

            # f path activations
            x1 = wk.tile([128, 512], BF16, tag="x1")
            nc.scalar.activation(x1[:], pf1[:], AF.Relu, bias=bfin2[:],
                                 scale=1.0)

            # support matmul: x_g1[i, (b,n)] = sum_m x[b,m,i] * A.T[m,n]
            # support pairs in two half-bank tiles (pair = 2 batches each)
            ps0 = psS.tile([64, 512], F32, tag="ps0")
            ps1 = psS.tile([64, 512], F32, tag="ps1")
            for pair, ps_ in ((0, ps0), (1, ps1)):
                for bb in range(2):
                    b = pair * 2 + bb
                    bs = slice(bb * 256, (bb + 1) * 256)
                    nc.tensor.matmul(ps_[:, bs],
                                     xt[:, b * 64:(b + 1) * 64],
                                     at0[:], start=True, stop=False)
                    nc.tensor.matmul(ps_[:, bs],
                                     xt[:, 256 + b * 64:256 + (b + 1) * 64],
                                     at1[:], start=False, stop=True)

            pf2 = psA.tile([128, 512], F32, tag="m")
            nc.tensor.matmul(pf2[:], wfhid[:], x1[:], start=True, stop=True)
            x2 = wk.tile([128, 512], BF16, tag="x2")
            nc.scalar.activation(x2[:], pf2[:], AF.Relu, bias=bfhid2[:],
                                 scale=1.0)

            # support evac: chunk0 via vector, chunk1 via scalar
            nc.vector.tensor_copy(XG[64:128, 0:512], ps0[:])
            nc.scalar.copy(XG[64:128, 512:1024], ps1[:])

            # ---- U / V / wgoutd pipeline + f-tail interleaved ----
            pgo0 = psG.tile([128, 512], F32, tag="go0")
            pgo1 = psG.tile([128, 512], F32, tag="go1")
            pgo = [pgo0, pgo1]
            pF = []
            Ff = wk.tile([128, 1024], BF16, tag="F")
            gfold = wk.tile([128, 1024], BF16, tag="G")
            # V engine assignment per (c): last c of each chunk goes via
            # scalar-copy + gpsimd-mask, rest direct on vector.
            for c2 in range(2):
                cs = slice(c2 * 512, (c2 + 1) * 512)
                for c in range(5):
                    pU = psU.tile([128, 512], F32, tag="u")
                    nc.tensor.matmul(pU[:], wp[:, c * 128:(c + 1) * 128],
                                     XG[:, cs], start=True, stop=True)
                    V = vp.tile([128, 512], BF16, tag="V")
                    if c == 3:
                        Uc = vp.tile([128, 512], BF16, tag="Uc")
                        nc.scalar.copy(Uc[:], pU[:])
                        nc.gpsimd.tensor_tensor(V[:], Uc[:], egt[c][:, cs],
                                                ALU.mult)
                    else:
                        nc.vector.tensor_tensor(V[:], pU[:], egt[c][:, cs],
                                                ALU.mult)
                    nc.tensor.matmul(pgo[c2][:], wgoutd[:], V[:],
                                     start=(c == 0), stop=False,
                                     skip_group_check=True)
                    # f-path filler matmuls early in chunk0
                    if c2 == 0 and c == 0:
                        pa = psA.tile([128, 512], F32, tag="m")
                        nc.tensor.matmul(pa[:], wfout_a[:], x2[:],
                                         start=True, stop=True)
                        pF.append(pa)
                    if c2 == 0 and c == 1:
                        pb = psA.tile([128, 512], F32, tag="m")
                        nc.tensor.matmul(pb[:], wfout_b[:], x2[:],
                                         start=True, stop=True)
                        pF.append(pb)
                        nc.scalar.activation(Ff[:, 0:512], pF[0][:], AF.Tanh,
                                             bias=bfout[:], scale=1.0)
                    if c2 == 0 and c == 2:
                        nc.scalar.activation(Ff[:, 512:1024], pF[1][:],
                                             AF.Tanh, bias=bfout[:], scale=1.0)
                nc.tensor.matmul(pgo[c2][:], bp2[:], egu[:, cs],
                                 start=False, stop=True, skip_group_check=True)
                nc.scalar.activation(gfold[:, cs], pgo[c2][:], AF.Tanh,
                                     bias=bgout[:], scale=1.0)

            # kh = sum_i F_i * dX_i ; kz = sum_i G_i * F_i * dX_i
            # (TT needs equal input base partitions -> copy upper halves
            # down to base 0 before the i-fold adds)
            khz = kp.tile([128, 1024], BF16, tag=k_tag)
            m01 = wk.tile([128, 1024], BF16, tag="m01")
            nc.vector.tensor_tensor(m01[:], Ff[:], dxf[:], ALU.mult)
            m1c = wk.tile([64, 1024], BF16, tag="m1c")
            nc.vector.tensor_copy(m1c[:], m01[64:128, :])
            n01 = wk.tile([128, 1024], BF16, tag="n01")
            nc.vector.tensor_tensor(n01[:], gfold[:], m01[:], ALU.mult)
            n1c = wk.tile([64, 1024], BF16, tag="n1c")
            nc.vector.tensor_copy(n1c[:], n01[64:128, :])
            for c2 in range(2):
                cs = slice(c2 * 512, (c2 + 1) * 512)
                os_ = slice(c2 * 64, (c2 + 1) * 64)
                nc.gpsimd.tensor_tensor(khz[os_, 0:512], m01[0:64, cs],
                                        m1c[:, cs], ALU.add)
                nc.gpsimd.tensor_tensor(khz[os_, 512:1024], n01[0:64, cs],
                                        n1c[:, cs], ALU.add)
            return khz

        third = 1.0 / 3.0
        for t in range(n_steps):
            k1 = vf(hz, _dx_stage_index(t, 0), "k1", wfinr, wginfr)

            # on-chain: u2 = hz + k1/3 (z-half on vector first)
            u2 = wk.tile([128, 1024], BF16, tag="u2")
            nc.vector.scalar_tensor_tensor(u2[:, 512:1024], k1[:, 512:1024],
                                           third, hz[:, 512:1024],
                                           ALU.mult, ALU.add)
            nc.vector.scalar_tensor_tensor(u2[:, 0:512], k1[:, 0:512],
                                           third, hz[:, 0:512],
                                           ALU.mult, ALU.add)
            # off-chain partials needing only k1 (r8/wf on the fp32
            # state-accumulation path stay fp32)
            w3 = wk.tile([128, 1024], BF16, tag="w3")
            nc.vector.scalar_tensor_tensor(w3[:], k1[:], -third, hz[:],
                                           ALU.mult, ALU.add)
            r8 = wk.tile([128, 1024], F32, tag="r8")
            nc.vector.scalar_tensor_tensor(r8[:], k1[:], 0.125, hz[:],
                                           ALU.mult, ALU.add)
            k2 = vf(u2, _dx_stage_index(t, 1), "k2", wfin, wginf)

            u3 = wk.tile([128, 1024], BF16, tag="u3")
            nc.vector.tensor_tensor(u3[:, 512:1024], k2[:, 512:1024],
                                    w3[:, 512:1024], ALU.add)
            nc.gpsimd.tensor_tensor(u3[:, 0:512], k2[:, 0:512], w3[:, 0:512],
                                    ALU.add)
            p12 = wk.tile([128, 1024], BF16, tag="p12")
            nc.vector.tensor_tensor(p12[:], k1[:], k2[:], ALU.subtract)
            w4 = wk.tile([128, 1024], BF16, tag="w4")
            nc.vector.tensor_tensor(w4[:], p12[:], hz[:], ALU.add)
            k3 = vf(u3, _dx_stage_index(t, 2), "k3", wfin, wginf)

            u4 = wk.tile([128, 1024], BF16, tag="u4")
            nc.vector.tensor_tensor(u4[:, 512:1024], k3[:, 512:1024],
                                    w4[:, 512:1024], ALU.add)
            nc.gpsimd.tensor_tensor(u4[:, 0:512], k3[:, 0:512], w4[:, 0:512],
                                    ALU.add)
            q23 = wk.tile([128, 1024], F32, tag="q23")
            nc.vector.tensor_tensor(q23[:], k2[:], k3[:], ALU.add)
            wf = wk.tile([128, 1024], F32, tag="wf")
            nc.vector.scalar_tensor_tensor(wf[:], q23[:], 0.375, r8[:],
                                           ALU.mult, ALU.add)
            k4 = vf(u4, _dx_stage_index(t, 3), "k4", wfin, wginf)

            # y' = wf + k4/8  (fp32 state)
            hzn = st.tile([128, 1024], F32R, tag="hz")
            nc.vector.scalar_tensor_tensor(hzn[:, 512:1024], k4[:, 512:1024],
                                           0.125, wf[:, 512:1024],
                                           ALU.mult, ALU.add)
            nc.vector.scalar_tensor_tensor(hzn[:, 0:512], k4[:, 0:512],
                                           0.125, wf[:, 0:512],
                                           ALU.mult, ALU.add)
            nc.sync.dma_start(ZOUT[t], hzn[:, 512:1024])
            hz = hzn

    nc.compile()
    return nc


def _fold(a):
    """[64, 1024] -> folded [128, 512]."""
    return np.concatenate([a[:, 0:512], a[:, 512:1024]], axis=0)


def _prep_shared(inputs):
    f32 = np.float32
    Eg = np.asarray(inputs["Eg"], f32)
    W_pool = np.asarray(inputs["W_pool"], f32)
    b_pool = np.asarray(inputs["b_pool"], f32)
    bg_in = np.asarray(inputs["bg_in"], f32)

    logits = Eg @ Eg.T
    r = np.maximum(logits, 0.0)
    e = np.exp(r - r.max(axis=1, keepdims=True))
    A = (e / e.sum(axis=1, keepdims=True)).astype(f32)
    AT = np.ascontiguousarray(A.T)

    WP = np.ascontiguousarray(
        np.transpose(W_pool, (1, 2, 0, 3)).reshape(KCH * HID, EMB * HID)
    ).astype(f32)

    n_of_tok = np.tile(np.arange(N), BLOC)
    EGU = np.ascontiguousarray(Eg.T[:, n_of_tok]).astype(f32)  # [10, 1024]
    EGT = np.empty((5, 128, TOK), f32)
    for c in range(5):
        for dd in range(2):
            EGT[c, dd * 64:(dd + 1) * 64, :] = Eg[n_of_tok, 2 * c + dd][None, :]

    # i-major permutation of the (HID, CIN)-reshaped output dims
    perm = np.empty(HID * CIN, np.int64)
    for i in range(CIN):
        for hh in range(HID):
            perm[i * HID + hh] = hh * CIN + i

    def bd(w):
        out = np.zeros((128, 128), f32)
        out[0:64, 0:64] = w
        out[64:128, 64:128] = w
        return out

    def halfpad(w, top):
        out = np.zeros((128, 128), f32)
        if top:
            out[0:64, :] = w
        else:
            out[64:128, :] = w
        return out

    Wf_out_p = np.asarray(inputs["Wf_out"], f32)[:, perm]
    bf_out_p = np.asarray(inputs["bf_out"], f32)[perm]
    Wg_out_p = np.asarray(inputs["Wg_out"], f32)[:, perm]
    bg_out_p = np.asarray(inputs["bg_out"], f32)[perm]

    Wg_in = np.asarray(inputs["Wg_in"], f32)

    b16 = lambda x: np.ascontiguousarray(x).astype(BF16NP)
    shared = {
        "WFIN": b16(bd(np.asarray(inputs["Wf_in"], f32))),
        "WFHID": b16(bd(np.asarray(inputs["Wf_hid"], f32))),
        "WGINF": b16(bd(Wg_in)),
        "WFINR": bd(np.asarray(inputs["Wf_in"], f32)),
        "WGINFR": bd(Wg_in),
        "WFOUT_A": b16(halfpad(Wf_out_p, True)),
        "WFOUT_B": b16(halfpad(Wf_out_p, False)),
        "WGOUTD": b16(np.concatenate([Wg_out_p, Wg_out_p], axis=0)),
        "BP2": b16(b_pool @ Wg_out_p),
        "BFIN2": np.tile(np.asarray(inputs["bf_in"], f32), 2)[:, None],
        "BFHID2": np.tile(np.asarray(inputs["bf_hid"], f32), 2)[:, None],
        "BGIN2": np.tile(bg_in, 2)[:, None].copy(),
        "BFOUT": bf_out_p[:, None].astype(f32).copy(),
        "BGOUT": bg_out_p[:, None].astype(f32).copy(),
        "IDENT": b16(np.eye(64, dtype=f32)),
        "AT0": b16(AT[0:128, :]),
        "AT1": b16(AT[128:256, :]),
        "WP": b16(WP),
        "EGU": b16(EGU),
    }
    for c in range(5):
        shared[f"EGT{c}"] = b16(EGT[c])
    return shared


def _prep_core(inputs, core, n_steps=NSTEP):
    f32 = np.float32
    ca = np.asarray(inputs["coeff_a"], f32)
    cb = np.asarray(inputs["coeff_b"], f32)
    cc = np.asarray(inputs["coeff_two_c"], f32)
    cd = np.asarray(inputs["coeff_three_d"], f32)
    W_h = np.asarray(inputs["W_h"], f32)
    b_h = np.asarray(inputs["b_h"], f32)
    W_z = np.asarray(inputs["W_z"], f32)
    b_z = np.asarray(inputs["b_z"], f32)

    bsl = slice(core * BLOC, (core + 1) * BLOC)
    x0 = ca[bsl, :, 0, :]                       # [4, 256, 2]
    h0 = (x0 @ W_h + b_h).reshape(TOK, HID).T   # [64, 1024]
    z0 = (x0 @ W_z + b_z).reshape(TOK, HID).T

    # 37 stage dX tensors; rows 0:64 = input chan 0 (bcast to 64
    # partitions), rows 64:128 = chan 1 -- i-major, matching F/G rows.
    DXB = np.empty((NSTAGE, 128, TOK), f32)
    maxidx = T - 2
    for si in range(NSTAGE):
        tt, s = si // 3, si % 3
        tval = tt + s / 3.0
        idx = min(int(np.floor(tval + 1e-9)), maxidx)
        frac = f32(tval - idx)
        dx = cb[bsl, :, idx, :] + (cc[bsl, :, idx, :]
                                   + cd[bsl, :, idx, :] * frac) * frac
        dx = dx.reshape(TOK, CIN)
        DXB[si, 0:64, :] = dx[:, 0][None, :]
        DXB[si, 64:128, :] = dx[:, 1][None, :]

    H0Z0 = np.concatenate([_fold(h0), _fold(z0)], axis=1)  # [128, 1024]
    return {
        "H0Z0": np.ascontiguousarray(H0Z0).astype(f32),
        "DXB": np.ascontiguousarray(DXB).astype(BF16NP),
    }, (x0 @ W_z + b_z)  # z0 unfolded [4, 256, 64] for output t=0


def kernel(**inputs):
    from concourse.bass_utils import run_bass_kernel_spmd

    n_steps = int(os.environ.get("GCDE_NSTEPS", NSTEP))
    key = n_steps
    if key not in _KERNEL_CACHE:
        _KERNEL_CACHE[key] = _build(n_steps)
    nc = _KERNEL_CACHE[key]

    shared = _prep_shared(inputs)
    in_maps = []
    z0_full = np.empty((B, N, HID), np.float32)
    for core in range(NCORES):
        per, z0c = _prep_core(inputs, core, n_steps)
        z0_full[core * BLOC:(core + 1) * BLOC] = z0c
        in_maps.append({**shared, **per})

    res = run_bass_kernel_spmd(nc, in_maps, list(range(NCORES)))
    global LAST_RES
    LAST_RES = res

    out = np.empty((B, N, T, HID), np.float32)
    out[:, :, 0, :] = z0_full
    for core in range(NCORES):
        Z = np.asarray(res.results[core]["ZOUT"][:n_steps],
                       dtype=np.float32)  # [n_steps, 128, 512]
        zt = np.concatenate([Z[:, 0:64, :], Z[:, 64:128, :]], axis=2)
        # zt: [n_steps, 64, 1024] -> [n_steps, 1024, 64] -> [.., 4, 256, 64]
        zt = zt.transpose(0, 2, 1).reshape(n_steps, BLOC, N, HID)
        for t in range(n_steps):
            out[core * BLOC:(core + 1) * BLOC, :, t + 1, :] = zt[t]
        if n_steps < NSTEP:
            out[:, :, n_steps + 1:, :] = 0.0
    return out


# revision 22
# speedup vs baseline: 1.2797x; 1.2797x over previous
"""NeuralGCDE Trainium2 kernel (bf16 rewrite).

Strategy: data-parallel over batch B=32 across 8 NeuronCores (B_loc=4 per
core, graph supports/weights replicated, zero inter-core communication).
Per core, the RK4 time scan (12 steps x 4 stages) runs fully on-device.

Layouts (per core, tokens tok = b*256+n, 1024 tokens, 2 chunks of 512):
  - state hz [128, 1024] bf16: partition p = 64*chunk + feature,
    cols 0:512 = h (token within chunk), cols 512:1024 = z.
  - XG [128 (k*64+i), 1024 tok]: graph-conv input (k=0: x, k=1: A@x)
  - xt[mi] [128 (node m within half mi), 4b*64+i]: token-major x for the
    support matmul, produced DIRECTLY from z by 8 small matmuls
    (stationary = z-slices, moving = Wg_in) -- no PE transposes.
  - adaptive per-node weights factorized through the embedding (EMB=10):
    U_c = wp_c.T @ XG; V_c = U_c .* Eg-mask (vector/gpsimd); the
    d-reduction and output projection fused into one accumulating
    matmul chain over WGOUTD; node bias via (b_pool@Wg_out).T @ EGU.
All compute bf16 (PSUM fp32); RK state kept in bf16 with one-op on-chain
updates (off-chain partial combos overlap the previous RK stage).
"""
import sys
import os
import numpy as np
import ml_dtypes

if "/opt/trn_rl_repo" not in sys.path:
    sys.path.insert(0, "/opt/trn_rl_repo")

BF16NP = ml_dtypes.bfloat16

B, N, T, CIN, HID, EMB, KCH = 32, 256, 13, 2, 64, 10, 2
NCORES = 8
BLOC = B // NCORES          # 4
TOK = BLOC * N              # 1024
NSTEP = T - 1               # 12
NSTAGE = 3 * NSTEP + 1      # 37 distinct spline-derivative tensors

_KERNEL_CACHE = {}
LAST_RES = None  # BassKernelResults of the most recent run (for test.py)


def _dx_stage_index(t, s):
    """Index into the 37-entry dX table for RK stage s of step t."""
    if s < 3:
        return 3 * t + s
    return 3 * (t + 1) if (t + 1) < NSTEP else 3 * NSTEP


def _build(n_steps=NSTEP):
    import concourse.bacc as bacc
    import concourse.tile as tile
    from concourse import mybir
    from contextlib import ExitStack

    F32 = mybir.dt.float32
    F32R = mybir.dt.float32r
    BF16 = mybir.dt.bfloat16
    AF = mybir.ActivationFunctionType
    ALU = mybir.AluOpType

    nc = bacc.Bacc("TRN2", target_bir_lowering=False, debug=False,
                   num_devices=NCORES)

    def din(name, shape, dt=BF16):
        return nc.dram_tensor(name, shape, dt, kind="ExternalInput").ap()

    H0Z0 = din("H0Z0", [128, 1024], mybir.dt.float32r)
    WFIN = din("WFIN", [128, 128])        # block-diag dup of Wf_in
    WFHID = din("WFHID", [128, 128])
    WGINF = din("WGINF", [128, 128])      # block-diag dup of Wg_in
    WFINR = din("WFINR", [128, 128], mybir.dt.float32r)
    WGINFR = din("WGINFR", [128, 128], mybir.dt.float32r)
    WFOUT_A = din("WFOUT_A", [128, 128])  # [Wf_out_perm; 0]
    WFOUT_B = din("WFOUT_B", [128, 128])  # [0; Wf_out_perm]
    WGOUTD = din("WGOUTD", [128, 128])    # [Wg_out_perm; Wg_out_perm]
    BP2 = din("BP2", [10, 128])           # b_pool @ Wg_out_perm
    BFIN2 = din("BFIN2", [128, 1], F32)
    BFHID2 = din("BFHID2", [128, 1], F32)
    BGIN2 = din("BGIN2", [128, 1], F32)
    BFOUT = din("BFOUT", [128, 1], F32)   # i-major permuted
    BGOUT = din("BGOUT", [128, 1], F32)
    IDENT = din("IDENT", [64, 64])
    AT0 = din("AT0", [128, 256])          # A.T rows 0:128
    AT1 = din("AT1", [128, 256])
    WP = din("WP", [128, 640])            # [k*64+i, d*64+o]
    EGU = din("EGU", [10, 1024])          # Eg[n(tok), d]
    EGT = [din(f"EGT{c}", [128, 1024]) for c in range(5)]
    DXB = din("DXB", [NSTAGE, 128, 1024])
    ZOUT = nc.dram_tensor("ZOUT", [NSTEP, 128, 512], mybir.dt.float32r,
                          kind="ExternalOutput").ap()

    with tile.TileContext(nc) as tc, ExitStack() as ctx:
        cp = ctx.enter_context(tc.tile_pool(name="const", bufs=1))
        st = ctx.enter_context(tc.tile_pool(name="state", bufs=2))
        kp = ctx.enter_context(tc.tile_pool(name="kpool", bufs=2))
        wk = ctx.enter_context(tc.tile_pool(name="work", bufs=2))
        vp = ctx.enter_context(tc.tile_pool(name="vpool", bufs=3))
        dxp = ctx.enter_context(tc.tile_pool(name="dxp", bufs=3))
        psA = ctx.enter_context(tc.tile_pool(name="psA", bufs=2, space="PSUM"))
        psXT = ctx.enter_context(tc.tile_pool(name="psXT", bufs=1, space="PSUM"))
        psS = ctx.enter_context(tc.tile_pool(name="psS", bufs=1, space="PSUM"))
        psU = ctx.enter_context(tc.tile_pool(name="psU", bufs=2, space="PSUM"))
        psG = ctx.enter_context(tc.tile_pool(name="psG", bufs=1, space="PSUM"))

        # ---- resident constants ----
        def cload(src, shape, tag, dt=BF16):
            t = cp.tile(shape, dt, tag=tag)
            nc.sync.dma_start(t[:], src)
            return t

        wfin = cload(WFIN, [128, 128], "wfin")
        wfhid = cload(WFHID, [128, 128], "wfhid")
        wginf = cload(WGINF, [128, 128], "wginf")
        wfinr = cload(WFINR, [128, 128], "wfinr", F32R)
        wginfr = cload(WGINFR, [128, 128], "wginfr", F32R)
        wfout_a = cload(WFOUT_A, [128, 128], "wfout_a")
        wfout_b = cload(WFOUT_B, [128, 128], "wfout_b")
        wgoutd = cload(WGOUTD, [128, 128], "wgoutd")
        bp2 = cload(BP2, [10, 128], "bp2")
        bfin2 = cload(BFIN2, [128, 1], "bfin2", F32)
        bfhid2 = cload(BFHID2, [128, 1], "bfhid2", F32)
        bgin2 = cload(BGIN2, [128, 1], "bgin2", F32)
        bfout = cload(BFOUT, [128, 1], "bfout", F32)
        bgout = cload(BGOUT, [128, 1], "bgout", F32)
        ident = cload(IDENT, [64, 64], "ident")
        at0 = cload(AT0, [128, 256], "at0")
        at1 = cload(AT1, [128, 256], "at1")
        wp = cload(WP, [128, 640], "wp")
        egu = cload(EGU, [10, 1024], "egu")
        egt = [cload(EGT[c], [128, 1024], f"egt{c}") for c in range(5)]

        # ---- state ----
        hz = st.tile([128, 1024], F32R, tag="hz")
        nc.sync.dma_start(hz[:], H0Z0)

        def vf(hzin, sidx, k_tag, head_f, head_g):
            """One vector-field eval -> khz [128,1024] bf16 (kh|kz)."""
            dxf = dxp.tile([128, 1024], BF16, tag="dxf")
            nc.sync.dma_start(dxf[:], DXB[sidx])

            # ---- PE head: pg1, pf1 ----
            pg1 = psA.tile([128, 512], F32, tag="m")
            nc.tensor.matmul(pg1[:], head_g[:], hzin[:, 512:1024],
                             start=True, stop=True)
            pf1 = psA.tile([128, 512], F32, tag="m")
            nc.tensor.matmul(pf1[:], head_f[:], hzin[:, 0:512],
                             start=True, stop=True)

            # XG rows 0:64 = x = relu(pg1 + bg_in)  (scalar + vector halves)
            XG = wk.tile([128, 1024], BF16, tag="XG")
            nc.scalar.activation(XG[0:64, 0:512], pg1[0:64, :], AF.Relu,
                                 bias=bgin2[0:64], scale=1.0)
            nc.vector.tensor_scalar(XG[0:64, 512:1024], pg1[64:128, :],
                                    bgin2[64:128], 0.0, ALU.add, ALU.max)

            # xt via PE transposes of x: xt cols mi*256 + b*64 + i
            pxt = psXT.tile([128, 512], BF16, tag="pxt")
            for mi in range(2):
                for b in range(BLOC):
                    nc.tensor.transpose(
                        pxt[:, mi * 256 + b * 64: mi * 256 + (b + 1) * 64],
                        XG[0:64, b * 256 + mi * 128: b * 256 + (mi + 1) * 128],
                        ident[:],
                    )
            xt = wk.tile([128, 512], BF16, tag="xt")
            nc.vector.tensor_copy(xt[:, 0:256], pxt[:, 0:256])
            nc.scalar.copy(xt[:, 256:512], pxt[:, 256:512])

            # f path activations
            x1 = wk.tile([128, 512], BF16, tag="x1")
            nc.scalar.activation(x1[:], pf1[:], AF.Relu, bias=bfin2[:],
                                 scale=1.0)

            # support matmul: x_g1[i, (b,n)] = sum_m x[b,m,i] * A.T[m,n]
            # support pairs in two half-bank tiles (pair = 2 batches each)
            ps0 = psS.tile([64, 512], F32, tag="ps0")
            ps1 = psS.tile([64, 512], F32, tag="ps1")
            for pair, ps_ in ((0, ps0), (1, ps1)):
                for bb in range(2):
                    b = pair * 2 + bb
                    bs = slice(bb * 256, (bb + 1) * 256)
                    nc.tensor.matmul(ps_[:, bs],
                                     xt[:, b * 64:(b + 1) * 64],
                                     at0[:], start=True, stop=False)
                    nc.tensor.matmul(ps_[:, bs],
                                     xt[:, 256 + b * 64:256 + (b + 1) * 64],
                                     at1[:], start=False, stop=True)

            pf2 = psA.tile([128, 512], F32, tag="m")
            nc.tensor.matmul(pf2[:], wfhid[:], x1[:], start=True, stop=True)
            x2 = wk.tile([128, 512], BF16, tag="x2")
            nc.scalar.activation(x2[:], pf2[:], AF.Relu, bias=bfhid2[:],
                                 scale=1.0)

            # support evac: chunk0 via vector, chunk1 via scalar
            nc.vector.tensor_copy(XG[64:128, 0:512], ps0[:])
            nc.scalar.copy(XG[64:128, 512:1024], ps1[:])

            # ---- U / V / wgoutd pipeline + f-tail interleaved ----
            pgo = []
            pF = []
            Ff = wk.tile([128, 1024], BF16, tag="F")
            gfold = wk.tile([128, 1024], BF16, tag="G")
            # V engine assignment per (c): last c of each chunk goes via
            # scalar-copy + gpsimd-mask, rest direct on vector.
            for c2 in range(2):
                cs = slice(c2 * 512, (c2 + 1) * 512)
                pgoc = psG.tile([128, 512], F32, tag="go")
                pgo.append(pgoc)
                Vs = []
                pUs = []

                def emit_u(c):
                    pU = psU.tile([128, 512], F32, tag="u")
                    nc.tensor.matmul(pU[:], wp[:, c * 128:(c + 1) * 128],
                                     XG[:, cs], start=True, stop=True)
                    pUs.append(pU)

                def emit_v(c):
                    pU = pUs[c]
                    V = vp.tile([128, 512], BF16, tag="V")
                    if c == 3:
                        Uc = vp.tile([128, 512], BF16, tag="Uc")
                        nc.scalar.copy(Uc[:], pU[:])
                        nc.gpsimd.tensor_tensor(V[:], Uc[:], egt[c][:, cs],
                                                ALU.mult)
                    else:
                        nc.vector.tensor_tensor(V[:], pU[:], egt[c][:, cs],
                                                ALU.mult)
                    Vs.append(V)

                def emit_wg(c):
                    nc.tensor.matmul(pgo[c2][:], wgoutd[:], Vs[c][:],
                                     start=(c == 0), stop=False,
                                     skip_group_check=True)

                # software-pipelined: U(c+1) issues before wg(c)
                emit_u(0)
                emit_v(0)
                emit_u(1)
                emit_v(1)
                emit_wg(0)
                emit_u(2)
                emit_v(2)
                emit_wg(1)
                emit_u(3)
                emit_v(3)
                emit_wg(2)
                emit_u(4)
                emit_v(4)
                emit_wg(3)
                if c2 == 0:
                    pa = psA.tile([128, 512], F32, tag="m")
                    nc.tensor.matmul(pa[:], wfout_a[:], x2[:],
                                     start=True, stop=True)
                    pb = psA.tile([128, 512], F32, tag="m")
                    nc.tensor.matmul(pb[:], wfout_b[:], x2[:],
                                     start=True, stop=True)
                    pF.extend((pa, pb))
                    nc.scalar.activation(Ff[:, 0:512], pF[0][:], AF.Tanh,
                                         bias=bfout[:], scale=1.0)
                    nc.scalar.activation(Ff[:, 512:1024], pF[1][:],
                                         AF.Tanh, bias=bfout[:], scale=1.0)
                emit_wg(4)
                nc.tensor.matmul(pgo[c2][:], bp2[:], egu[:, cs],
                                 start=False, stop=True, skip_group_check=True)
                nc.scalar.activation(gfold[:, cs], pgo[c2][:], AF.Tanh,
                                     bias=bgout[:], scale=1.0)

            # kh = sum_i F_i * dX_i ; kz = sum_i G_i * F_i * dX_i
            # (TT needs equal input base partitions -> copy upper halves
            # down to base 0 before the i-fold adds)
            khz = kp.tile([128, 1024], BF16, tag=k_tag)
            m01 = wk.tile([128, 1024], BF16, tag="m01")
            nc.vector.tensor_tensor(m01[:], Ff[:], dxf[:], ALU.mult)
            m1c = wk.tile([64, 1024], BF16, tag="m1c")
            nc.vector.tensor_copy(m1c[:], m01[64:128, :])
            n01 = wk.tile([128, 1024], BF16, tag="n01")
            nc.vector.tensor_tensor(n01[:], gfold[:], m01[:], ALU.mult)
            n1c = wk.tile([64, 1024], BF16, tag="n1c")
            nc.vector.tensor_copy(n1c[:], n01[64:128, :])
            for c2 in range(2):
                cs = slice(c2 * 512, (c2 + 1) * 512)
                os_ = slice(c2 * 64, (c2 + 1) * 64)
                nc.gpsimd.tensor_tensor(khz[os_, 0:512], m01[0:64, cs],
                                        m1c[:, cs], ALU.add)
                nc.gpsimd.tensor_tensor(khz[os_, 512:1024], n01[0:64, cs],
                                        n1c[:, cs], ALU.add)
            return khz

        third = 1.0 / 3.0
        for t in range(n_steps):
            k1 = vf(hz, _dx_stage_index(t, 0), "k1", wfinr, wginfr)

            # on-chain: u2 = hz + k1/3 (z-half on vector first)
            u2 = wk.tile([128, 1024], BF16, tag="u2")
            nc.vector.scalar_tensor_tensor(u2[:, 512:1024], k1[:, 512:1024],
                                           third, hz[:, 512:1024],
                                           ALU.mult, ALU.add)
            nc.vector.scalar_tensor_tensor(u2[:, 0:512], k1[:, 0:512],
                                           third, hz[:, 0:512],
                                           ALU.mult, ALU.add)
            # off-chain partials needing only k1 (r8/wf on the fp32
            # state-accumulation path stay fp32)
            w3 = wk.tile([128, 1024], BF16, tag="w3")
            nc.vector.scalar_tensor_tensor(w3[:], k1[:], -third, hz[:],
                                           ALU.mult, ALU.add)
            r8 = wk.tile([128, 1024], F32, tag="r8")
            nc.vector.scalar_tensor_tensor(r8[:], k1[:], 0.125, hz[:],
                                           ALU.mult, ALU.add)
            k2 = vf(u2, _dx_stage_index(t, 1), "k2", wfin, wginf)

            u3 = wk.tile([128, 1024], BF16, tag="u3")
            nc.vector.tensor_tensor(u3[:, 512:1024], k2[:, 512:1024],
                                    w3[:, 512:1024], ALU.add)
            nc.gpsimd.tensor_tensor(u3[:, 0:512], k2[:, 0:512], w3[:, 0:512],
                                    ALU.add)
            p12 = wk.tile([128, 1024], BF16, tag="p12")
            nc.vector.tensor_tensor(p12[:], k1[:], k2[:], ALU.subtract)
            w4 = wk.tile([128, 1024], BF16, tag="w4")
            nc.vector.tensor_tensor(w4[:], p12[:], hz[:], ALU.add)
            k3 = vf(u3, _dx_stage_index(t, 2), "k3", wfin, wginf)

            u4 = wk.tile([128, 1024], BF16, tag="u4")
            nc.vector.tensor_tensor(u4[:, 512:1024], k3[:, 512:1024],
                                    w4[:, 512:1024], ALU.add)
            nc.gpsimd.tensor_tensor(u4[:, 0:512], k3[:, 0:512], w4[:, 0:512],
                                    ALU.add)
            q23 = wk.tile([128, 1024], F32, tag="q23")
            nc.vector.tensor_tensor(q23[:], k2[:], k3[:], ALU.add)
            wf = wk.tile([128, 1024], F32, tag="wf")
            nc.vector.scalar_tensor_tensor(wf[:], q23[:], 0.375, r8[:],
                                           ALU.mult, ALU.add)
            k4 = vf(u4, _dx_stage_index(t, 3), "k4", wfin, wginf)

            # y' = wf + k4/8  (fp32 state)
            hzn = st.tile([128, 1024], F32R, tag="hz")
            nc.vector.scalar_tensor_tensor(hzn[:, 512:1024], k4[:, 512:1024],
                                           0.125, wf[:, 512:1024],
                                           ALU.mult, ALU.add)
            nc.vector.scalar_tensor_tensor(hzn[:, 0:512], k4[:, 0:512],
                                           0.125, wf[:, 0:512],
                                           ALU.mult, ALU.add)
            nc.sync.dma_start(ZOUT[t], hzn[:, 512:1024])
            hz = hzn

    nc.compile()
    return nc


def _fold(a):
    """[64, 1024] -> folded [128, 512]."""
    return np.concatenate([a[:, 0:512], a[:, 512:1024]], axis=0)


def _prep_shared(inputs):
    f32 = np.float32
    Eg = np.asarray(inputs["Eg"], f32)
    W_pool = np.asarray(inputs["W_pool"], f32)
    b_pool = np.asarray(inputs["b_pool"], f32)
    bg_in = np.asarray(inputs["bg_in"], f32)

    logits = Eg @ Eg.T
    r = np.maximum(logits, 0.0)
    e = np.exp(r - r.max(axis=1, keepdims=True))
    A = (e / e.sum(axis=1, keepdims=True)).astype(f32)
    AT = np.ascontiguousarray(A.T)

    WP = np.ascontiguousarray(
        np.transpose(W_pool, (1, 2, 0, 3)).reshape(KCH * HID, EMB * HID)
    ).astype(f32)

    n_of_tok = np.tile(np.arange(N), BLOC)
    EGU = np.ascontiguousarray(Eg.T[:, n_of_tok]).astype(f32)  # [10, 1024]
    EGT = np.empty((5, 128, TOK), f32)
    for c in range(5):
        for dd in range(2):
            EGT[c, dd * 64:(dd + 1) * 64, :] = Eg[n_of_tok, 2 * c + dd][None, :]

    # i-major permutation of the (HID, CIN)-reshaped output dims
    perm = np.empty(HID * CIN, np.int64)
    for i in range(CIN):
        for hh in range(HID):
            perm[i * HID + hh] = hh * CIN + i

    def bd(w):
        out = np.zeros((128, 128), f32)
        out[0:64, 0:64] = w
        out[64:128, 64:128] = w
        return out

    def halfpad(w, top):
        out = np.zeros((128, 128), f32)
        if top:
            out[0:64, :] = w
        else:
            out[64:128, :] = w
        return out

    Wf_out_p = np.asarray(inputs["Wf_out"], f32)[:, perm]
    bf_out_p = np.asarray(inputs["bf_out"], f32)[perm]
    Wg_out_p = np.asarray(inputs["Wg_out"], f32)[:, perm]
    bg_out_p = np.asarray(inputs["bg_out"], f32)[perm]

    Wg_in = np.asarray(inputs["Wg_in"], f32)

    b16 = lambda x: np.ascontiguousarray(x).astype(BF16NP)
    shared = {
        "WFIN": b16(bd(np.asarray(inputs["Wf_in"], f32))),
        "WFHID": b16(bd(np.asarray(inputs["Wf_hid"], f32))),
        "WGINF": b16(bd(Wg_in)),
        "WFINR": bd(np.asarray(inputs["Wf_in"], f32)),
        "WGINFR": bd(Wg_in),
        "WFOUT_A": b16(halfpad(Wf_out_p, True)),
        "WFOUT_B": b16(halfpad(Wf_out_p, False)),
        "WGOUTD": b16(np.concatenate([Wg_out_p, Wg_out_p], axis=0)),
        "BP2": b16(b_pool @ Wg_out_p),
        "BFIN2": np.tile(np.asarray(inputs["bf_in"], f32), 2)[:, None],
        "BFHID2": np.tile(np.asarray(inputs["bf_hid"], f32), 2)[:, None],
        "BGIN2": np.tile(bg_in, 2)[:, None].copy(),
        "BFOUT": bf_out_p[:, None].astype(f32).copy(),
        "BGOUT": bg_out_p[:, None].astype(f32).copy(),
        "IDENT": b16(np.eye(64, dtype=f32)),
        "AT0": b16(AT[0:128, :]),
        "AT1": b16(AT[128:256, :]),
        "WP": b16(WP),
        "EGU": b16(EGU),
    }
    for c in range(5):
        shared[f"EGT{c}"] = b16(EGT[c])
    return shared


def _prep_core(inputs, core, n_steps=NSTEP):
    f32 = np.float32
    ca = np.asarray(inputs["coeff_a"], f32)
    cb = np.asarray(inputs["coeff_b"], f32)
    cc = np.asarray(inputs["coeff_two_c"], f32)
    cd = np.asarray(inputs["coeff_three_d"], f32)
    W_h = np.asarray(inputs["W_h"], f32)
    b_h = np.asarray(inputs["b_h"], f32)
    W_z = np.asarray(inputs["W_z"], f32)
    b_z = np.asarray(inputs["b_z"], f32)

    bsl = slice(core * BLOC, (core + 1) * BLOC)
    x0 = ca[bsl, :, 0, :]                       # [4, 256, 2]
    h0 = (x0 @ W_h + b_h).reshape(TOK, HID).T   # [64, 1024]
    z0 = (x0 @ W_z + b_z).reshape(TOK, HID).T

    # 37 stage dX tensors; rows 0:64 = input chan 0 (bcast to 64
    # partitions), rows 64:128 = chan 1 -- i-major, matching F/G rows.
    DXB = np.empty((NSTAGE, 128, TOK), f32)
    maxidx = T - 2
    for si in range(NSTAGE):
        tt, s = si // 3, si % 3
        tval = tt + s / 3.0
        idx = min(int(np.floor(tval + 1e-9)), maxidx)
        frac = f32(tval - idx)
        dx = cb[bsl, :, idx, :] + (cc[bsl, :, idx, :]
                                   + cd[bsl, :, idx, :] * frac) * frac
        dx = dx.reshape(TOK, CIN)
        DXB[si, 0:64, :] = dx[:, 0][None, :]
        DXB[si, 64:128, :] = dx[:, 1][None, :]

    H0Z0 = np.concatenate([_fold(h0), _fold(z0)], axis=1)  # [128, 1024]
    return {
        "H0Z0": np.ascontiguousarray(H0Z0).astype(f32),
        "DXB": np.ascontiguousarray(DXB).astype(BF16NP),
    }, (x0 @ W_z + b_z)  # z0 unfolded [4, 256, 64] for output t=0


def kernel(**inputs):
    from concourse.bass_utils import run_bass_kernel_spmd

    n_steps = int(os.environ.get("GCDE_NSTEPS", NSTEP))
    key = n_steps
    if key not in _KERNEL_CACHE:
        _KERNEL_CACHE[key] = _build(n_steps)
    nc = _KERNEL_CACHE[key]

    shared = _prep_shared(inputs)
    in_maps = []
    z0_full = np.empty((B, N, HID), np.float32)
    for core in range(NCORES):
        per, z0c = _prep_core(inputs, core, n_steps)
        z0_full[core * BLOC:(core + 1) * BLOC] = z0c
        in_maps.append({**shared, **per})

    res = run_bass_kernel_spmd(nc, in_maps, list(range(NCORES)))
    global LAST_RES
    LAST_RES = res

    out = np.empty((B, N, T, HID), np.float32)
    out[:, :, 0, :] = z0_full
    for core in range(NCORES):
        Z = np.asarray(res.results[core]["ZOUT"][:n_steps],
                       dtype=np.float32)  # [n_steps, 128, 512]
        zt = np.concatenate([Z[:, 0:64, :], Z[:, 64:128, :]], axis=2)
        # zt: [n_steps, 64, 1024] -> [n_steps, 1024, 64] -> [.., 4, 256, 64]
        zt = zt.transpose(0, 2, 1).reshape(n_steps, BLOC, N, HID)
        for t in range(n_steps):
            out[core * BLOC:(core + 1) * BLOC, :, t + 1, :] = zt[t]
        if n_steps < NSTEP:
            out[:, :, n_steps + 1:, :] = 0.0
    return out
